# revision 50
# baseline (speedup 1.0000x reference)
"""Trainium2 Bass kernel for GroupedQueryAttention (v3).

Sharding: 8 cores; core c owns KV head g=c and Q heads 4c..4c+3, both batch
elements. Each core computes its [2, 2048, 256] output slice; host concats.

Host prep: hs is transposed to hsT [B, D, S] and cast to fp16 on the host;
1/sqrt(HD) is folded into Wq/bq; weights are cast to fp16. (fp16 storage is
speed-identical to bf16 on PE/ACT/DVE but carries 3 extra mantissa bits,
roughly halving the kernel's error vs the 2e-2 budget; the DVE Schraudolph
exp constants are retuned for the fp16 bit layout.)

Per-core dataflow:
  P) Projections: Q^T (2 tiles of [128, S], head pairs), [K^T|V^T] [128, S]
     accumulate over 16 d-tiles directly from hsT (no on-device transposes).
     Startup runs 8 concurrent accumulation groups (kv + q0 + q1 pair
     passes) across all 8 PSUM banks while the serialized hsT DMA streams
     in. K^T is duplicated at partitions 64:128 (kth) for odd heads; V^T
     tiles are PE-transposed back to natural [s_k, 64] + ones column -> v1.
  A) Attention per (b, h, s_q-chunk of 1024): scores computed transposed,
     S^T [s_k=128, s_q=1024] per k-tile; exp mostly on ACT (bf16 out), a
     greedily-paced minority of k-tiles on DVE via a Schraudolph int16 bit
     trick (GPSIMD has no PSUM port); PV in natural orientation:
     ctx[s_q-block, 65] accumulates ex_chunk^T @ [V|1] over k-tiles in PSUM
     (ones column = softmax denominator); DVE does the reciprocal-scale
     epilogue into a per-batch bf16 output tile (host casts back to f32).
  Remaining projection matmuls are interleaved as paced filler inside the
  attention k-loop (with completion markers + requirement drains), keeping
  the PE continuously busy so it holds the 2.4 GHz p-state. Once the filler
  drains, the projection PSUM banks are handed to a third scores buffer,
  deepening the scores->exp->scores pipeline for the tail.

All matmul moving operands are bf16 (1 col/cycle at any output width).
"""

import sys
from collections import deque
from contextlib import ExitStack

import numpy as np

sys.path.insert(0, "/opt/trn_rl_repo")

import concourse.bass as bass  # noqa: E402
import concourse.bacc as bacc  # noqa: E402
import concourse.tile as tile  # noqa: E402
from concourse import mybir  # noqa: E402
from concourse.bass_utils import run_bass_kernel_spmd  # noqa: E402

B = 2
S = 2048
D = 2048
HD = 64
NCORES = 8
QH = 4           # q heads per core
MCOLS = QH * HD  # 256 output cols per core

BF16 = mybir.dt.float16  # fp16 storage everywhere (same speed, 8x mantissa)
F32 = mybir.dt.float32
I16 = mybir.dt.int16
Exp = mybir.ActivationFunctionType.Exp
Mult = mybir.AluOpType.mult
Add = mybir.AluOpType.add

NDT = 16         # d tiles of 128
NSC = 4          # s chunks of 512 (projection)
NKT = 16         # s_k tiles of 128
NSQ = 2          # s_q chunks of 1024

# Schraudolph exp constants (fp16-as-int16; tuned for truncating convert)
SCH_A = 1024.0 / 0.6931471805599453
SCH_B = 15360.0 - 44.0
# Max exp tiles offloaded to DVE via Schraudolph (of 256). fp16 storage of
# the exact-exp path frees error budget, so the cap can sit near the
# ACT/DVE balance point rather than the old bf16 error-driven 56.
# (GPSIMD/Pool has no PSUM port, so the offload engine must be DVE.)
DVE_EXP_CAP = 112
import os  # noqa: E402
ENABLE_STP2 = os.environ.get("K_STP2", "1") == "1"
ENABLE_KV_PAR = os.environ.get("K_KVPAR", "1") == "1"
K_DVECAP = int(os.environ.get("K_DVECAP", str(DVE_EXP_CAP)))
K_BUDGET = float(os.environ.get("K_BUDGET", "0.3"))
K_BIAS = os.environ.get("K_BIAS", "dve")      # filler bias engine
K_EPI = os.environ.get("K_EPI", "inline")     # defer | inline
K_SPLIT = os.environ.get("K_SPLIT", "greedy")  # greedy | static
K_EPIMUL = os.environ.get("K_EPIMUL", "alt")  # alt | dve (deferred muls)
K_LASTLAG = int(os.environ.get("K_LASTLAG", "4"))  # pv lag in final unit
K_EXPSPLIT = os.environ.get("K_EXPSPLIT", "0") == "1"
K_PVADAPT = os.environ.get("K_PVADAPT", "0") == "1"
K_PAIR = os.environ.get("K_PAIR", "0") == "1"
K_SUSPLIT = int(os.environ.get("K_SUSPLIT", "0"))
K_RUSHSPLIT = os.environ.get("K_RUSHSPLIT", "0") == "1"


def build_nc():
    nc = bacc.Bacc("TRN2", target_bir_lowering=False, debug=False)

    hst_d = nc.dram_tensor("hst", [B, D, S], BF16, kind="ExternalInput")
    wq_d = nc.dram_tensor("wq", [D, MCOLS], BF16, kind="ExternalInput")
    wkv_d = nc.dram_tensor("wkv", [D, 128], BF16, kind="ExternalInput")
    bq_d = nc.dram_tensor("bq", [128, 2], F32, kind="ExternalInput")
    bkv_d = nc.dram_tensor("bkv", [128, 1], F32, kind="ExternalInput")
    id_d = nc.dram_tensor("ident", [128, 128], BF16, kind="ExternalInput")
    out_d = nc.dram_tensor("out", [B, S, MCOLS], BF16, kind="ExternalOutput")

    with tile.TileContext(nc) as tc, ExitStack() as ctx:
        const = ctx.enter_context(tc.tile_pool(name="const", bufs=1))
        wqp = ctx.enter_context(tc.tile_pool(name="wqp", bufs=4))
        wkvp = ctx.enter_context(tc.tile_pool(name="wkvp", bufs=1))
        hstp = ctx.enter_context(tc.tile_pool(name="hstp", bufs=13))
        qtp = ctx.enter_context(tc.tile_pool(name="qtp", bufs=4))
        kvp = ctx.enter_context(tc.tile_pool(name="kvp", bufs=2))
        kthp = ctx.enter_context(tc.tile_pool(name="kthp", bufs=2))
        v1p = ctx.enter_context(tc.tile_pool(name="v1p", bufs=2 * NKT))
        expp = ctx.enter_context(tc.tile_pool(name="expp", bufs=5))
        recp = ctx.enter_context(tc.tile_pool(name="recp", bufs=4))
        outp = ctx.enter_context(tc.tile_pool(name="outp", bufs=2))
        if K_PAIR:
            # one [128, 2048] PSUM tile = both score slots, PSUM-contiguous,
            # so one merged exp instruction can read a k-tile PAIR: ACT cost
            # 1891ns per pair = 946ns/tile, below the ~970ns/kt PE cadence
            # (single [128,1024] exp costs 1038 and ACT falls behind)
            stp = ctx.enter_context(tc.tile_pool(name="stp", bufs=1,
                                                 space="PSUM"))
        else:
            stp = ctx.enter_context(tc.tile_pool(name="stp", bufs=2,
                                                 space="PSUM"))
        pvp = ctx.enter_context(tc.tile_pool(name="pvp", bufs=2, space="PSUM"))
        # projp's 2 banks are handed over to a third scores buffer (stp2)
        # once all projection work has been emitted — the deeper scores
        # pipeline hides the scores->exp->scores latency chain in the tail
        projp_cm = tc.tile_pool(name="projp", bufs=2, space="PSUM")
        projp = projp_cm.__enter__()
        psum_state = {"projp_cm": projp_cm, "stp2": None}

        # All DMA transfers serialize on one HWDGE device in the cost model,
        # in dispatch order — so put everything on one queue in exactly the
        # order the startup consumes it: (wkv_dt, hsT0_dt) pairs gate the kv
        # passes, then wq lands just in time for the q0 pass, then hsT b1.
        # consolidated loads: HWDGE charges ~630ns per DMA instruction, so
        # batch the weight/activation streams into few wide transfers
        # startup-latency split (K_SUSPLIT): dispatch small first slices of
        # wkv (and optionally hsT dt0) so the first kv matmul starts earlier;
        # each split costs one extra serialized HWDGE dispatch (~630ns)
        hsT = {}
        wkv_big = wkvp.tile([128, 16 * 128], BF16, tag="wkv", name="wkvbig")
        if K_SUSPLIT >= 1:
            nc.sync.dma_start(out=wkv_big[:, 0:128], in_=wkv_d[0:128, :])
            if K_SUSPLIT >= 2:
                t0 = hstp.tile([128, 2 * S], BF16, tag="hst", name="hsT0_0")
                nc.sync.dma_start(out=t0[:, 0:S], in_=hst_d[0, 0:128, :])
            nc.sync.dma_start(
                out=wkv_big[:, 128:].rearrange("p (blk c) -> p blk c", c=128),
                in_=wkv_d[128:, :].rearrange("(blk p) c -> p blk c", p=128))
            if K_SUSPLIT >= 2:
                nc.sync.dma_start(out=t0[:, S:2 * S], in_=hst_d[0, 128:256, :])
                hsT[(0, 0)] = t0[:, 0:S]
                hsT[(0, 1)] = t0[:, S:2 * S]
        else:
            nc.sync.dma_start(
                out=wkv_big[:].rearrange("p (blk c) -> p blk c", c=128),
                in_=wkv_d[:].rearrange("(blk p) c -> p blk c", p=128))
        wkv_sb = [wkv_big[:, dt_ * 128:(dt_ + 1) * 128] for dt_ in range(NDT)]

        wq_sb = []
        wq4 = []
        for q4 in range(4):
            wt = wqp.tile([128, 4 * MCOLS], BF16, tag="wq", name=f"wq4_{q4}")
            wq4.append(wt)
        for dt_ in range(NDT):
            if dt_ % 2 == 0 and (dt_ >= 2 or (0, 0) not in hsT):
                t = hstp.tile([128, 2 * S], BF16, tag="hst",
                              name=f"hsT0_{dt_}")
                nc.sync.dma_start(
                    out=t[:].rearrange("p (two s) -> p two s", s=S),
                    in_=hst_d[0, dt_ * 128:(dt_ + 2) * 128, :].rearrange(
                        "(two p) s -> p two s", p=128))
                hsT[(0, dt_)] = t[:, 0:S]
                hsT[(0, dt_ + 1)] = t[:, S:2 * S]
            if dt_ % 4 == 3:
                q4i = dt_ // 4
                nc.sync.dma_start(
                    out=wq4[q4i][:].rearrange("p (blk c) -> p blk c", c=MCOLS),
                    in_=wq_d[q4i * 512:(q4i + 1) * 512, :].rearrange(
                        "(blk p) c -> p blk c", p=128))
        wq_sb = [wq4[dt_ // 4][:, (dt_ % 4) * MCOLS:(dt_ % 4 + 1) * MCOLS]
                 for dt_ in range(NDT)]

        # consts dispatch from the scalar queue so the sync queue's first
        # dispatches are the startup-critical wkv/hsT tiles
        ident = const.tile([128, 128], BF16, tag="ident")
        nc.scalar.dma_start(out=ident[:], in_=id_d[:])
        bq_sb = const.tile([128, 2], F32, tag="bq")
        nc.scalar.dma_start(out=bq_sb[:], in_=bq_d[:])
        bkv_sb = const.tile([128, 1], F32, tag="bkv")
        nc.scalar.dma_start(out=bkv_sb[:], in_=bkv_d[:])
        zb = const.tile([128, 1], F32, tag="zb")
        nc.vector.memset(zb[:], 0.0)


        qT = {}   # (b, pair) -> [128, S] bf16
        kvT = {}  # b -> [128, S] bf16 (rows 0:64 K^T, 64:128 V^T)
        kth = {}  # b -> [128, S] bf16 (rows 64:128 K^T copy)
        v1 = {}   # (b, kt) -> [128, 65] bf16 ([V | 1])
        for b in range(B):
            kvT[b] = kvp.tile([128, S], BF16, tag="kv", name=f"kvT{b}")
            kth[b] = kthp.tile([128, S], BF16, tag="kth", name=f"kth{b}")
            for pair in range(2):
                qT[(b, pair)] = qtp.tile([128, S], BF16, tag="qt",
                                         name=f"qT{b}_{pair}")

        # ---- projection pass machinery ----
        # Steps are (weight, fn): weight ~ PE-engine cost in units of one
        # 512-wide matmul pair (427 ns); the attention loop consumes ~1.0
        # of weight per k-tile so cheap steps get batched.
        Ident = mybir.ActivationFunctionType.Identity

        def proj_pass_steps(b, kind, sc0, sc1, pool=None, aps=None,
                            bias_eng="dve"):
            """One pair-pass: two accumulation groups (s-chunks sc0, sc1).
            kind: 'kv' or ('q', qc). aps: explicit PSUM APs to accumulate
            into (bank-disjoint halves of donor tiles). bias_eng='act' puts
            the PSUM->SBUF bias-add on the ACT engine."""
            if aps is not None:
                psA, psB = aps
            else:
                pool = pool or projp
                tg = {id(stp): "st", id(pvp): "pv"}.get(id(pool), "pj")
                psA = pool.tile([128, 512], F32, tag=tg, name="pjA")
                psB = pool.tile([128, 512], F32, tag=tg, name="pjB")
            for dt_ in range(NDT):
                def mm_step(dt_=dt_, psA=psA, psB=psB):
                    for ps, sc in ((psA, sc0), (psB, sc1)):
                        rhs = hsT[(b, dt_)][:, sc * 512:(sc + 1) * 512]
                        if kind == "kv":
                            lhsT = wkv_sb[dt_][:]
                        else:
                            qc = kind[1]
                            lhsT = wq_sb[dt_][:, qc * 128:(qc + 1) * 128]
                        nc.tensor.matmul(ps[:], lhsT, rhs,
                                         start=(dt_ == 0), stop=(dt_ == NDT - 1))
                yield (1.0, None, 0.0, (), mm_step)

            def bias_step():
                for ps, sc in ((psA, sc0), (psB, sc1)):
                    c0, c1 = sc * 512, (sc + 1) * 512
                    if kind == "kv":
                        out_ap, b_ap = kvT[b][:, c0:c1], bkv_sb[:]
                    else:
                        qc = kind[1]
                        out_ap, b_ap = (qT[(b, qc)][:, c0:c1],
                                        bq_sb[:, qc:qc + 1])
                    if bias_eng == "act":
                        nc.scalar.activation(out_ap, ps[:], Ident, bias=b_ap)
                    else:
                        nc.vector.tensor_scalar_add(out_ap, ps[:], b_ap)
            if kind == "kv":
                marks = (("kv", b, sc0), ("kv", b, sc1))
            else:
                marks = (("q", b, kind[1], sc0), ("q", b, kind[1], sc1))
            yield (0.2, bias_eng, 1224.0 if bias_eng == "act" else 1320.0,
                   marks, bias_step)

        def vt_steps(b, copy_eng="dve"):
            """PE-transpose V^T tiles to natural [s_k, 64] + ones column."""
            for kt in range(NKT):
                def step(kt=kt):
                    pst = projp.tile([128, 64], BF16, tag="pj")
                    nc.tensor.transpose(
                        pst[:], kvT[b][64:128, kt * 128:(kt + 1) * 128],
                        ident[64:128, 64:128])
                    v = v1p.tile([128, 65], BF16, tag="v1", name=f"v1_{b}_{kt}")
                    if copy_eng == "act":
                        nc.scalar.activation(
                            v[:, 0:64], pst[:],
                            mybir.ActivationFunctionType.Copy)
                    else:
                        nc.vector.tensor_copy(v[:, 0:64], pst[:])
                    nc.gpsimd.memset(v[:, 64:65], 1.0)
                    v1[(b, kt)] = v
                yield (0.2, copy_eng, 238.0 if copy_eng == "act" else 160.0,
                       (("v1", b, kt),), step)

        def kth_step(b):
            def step():
                nc.sync.dma_start(out=kth[b][64:128, :], in_=kvT[b][0:64, :])
            yield (0.1, None, 0.0, (("kth", b),), step)

        def proj_stream(b):
            yield from proj_pass_steps(b, "kv", 0, 1, bias_eng=K_BIAS)
            yield from proj_pass_steps(b, "kv", 2, 3, bias_eng=K_BIAS)
            yield from kth_step(b)
            yield from vt_steps(b)
            yield from proj_pass_steps(b, ("q", 0), 0, 1, bias_eng=K_BIAS)
            yield from proj_pass_steps(b, ("q", 0), 2, 3, bias_eng=K_BIAS)
            yield from proj_pass_steps(b, ("q", 1), 0, 1, bias_eng=K_BIAS)
            yield from proj_pass_steps(b, ("q", 1), 2, 3, bias_eng=K_BIAS)

        # ---- batch 0 minimal prologue: kv + vt + q0 cols 0:1024; the rest
        # feeds the attention loop as filler. The two kv pair-passes run
        # concurrently (kv23 borrows the still-idle scores pool's banks) so
        # both track the serialized hsT DMA stream. ----
        done = set()
        # 8 concurrent accumulation groups during the DMA-bound startup:
        # kv01 in projp, q0-01 in pvp, and kv23 + q1-01 packed into the two
        # stp slots (bank-disjoint halves of [128,1024] donor tiles)
        if K_PAIR:
            stpair = stp.tile([128, 2048], F32, tag="st", name="stpair")
            stA = stpair[:, 0:1024]
            stB = stpair[:, 1024:2048]
        else:
            stpair = None
            stA = stp.tile([128, 1024], F32, tag="st", name="stdonA")
            stB = stp.tile([128, 1024], F32, tag="st", name="stdonB")
        streams = [
            proj_pass_steps(0, "kv", 0, 1, bias_eng="act"),
            proj_pass_steps(0, "kv", 2, 3,
                            aps=(stA[:, 0:512], stB[:, 0:512]),
                            bias_eng="act"),
            proj_pass_steps(0, ("q", 0), 0, 1, pool=pvp),
            proj_pass_steps(0, ("q", 1), 0, 1,
                            aps=(stA[:, 512:1024], stB[:, 512:1024]),
                            bias_eng="act"),
        ]
        for steps in zip(*streams):
            for _, _, _, m, s in steps:
                s()
                done.update(m)
        for _, _, _, m, step in kth_step(0):
            step()
            done.update(m)
        for _, _, _, m, step in vt_steps(0):
            step()
            done.update(m)

        for dt_ in range(0, NDT, 2):
            t = hstp.tile([128, 2 * S], BF16, tag="hst", name=f"hsT1_{dt_}")
            nc.sync.dma_start(
                out=t[:].rearrange("p (two s) -> p two s", s=S),
                in_=hst_d[1, dt_ * 128:(dt_ + 2) * 128, :].rearrange(
                    "(two p) s -> p two s", p=128))
            hsT[(1, dt_)] = t[:, 0:S]
            hsT[(1, dt_ + 1)] = t[:, S:2 * S]

        filler = deque()
        filler.extend(proj_pass_steps(0, ("q", 0), 2, 3, bias_eng=K_BIAS))
        filler.extend(proj_pass_steps(0, ("q", 1), 2, 3, bias_eng=K_BIAS))
        filler.extend(proj_stream(1))

        def pop_filler():
            w, eng, eng_ns, marks, fn = filler.popleft()
            fn()
            done.update(marks)
            sched["pe"] += w * 427.0
            sched["filler_w"] -= w
            if eng_ns:
                sched[eng] = max(sched[eng], sched["pe"]) + eng_ns
            return w

        def require(reqs):
            while filler and not all(r in done for r in reqs):
                pop_filler()

        # Greedy per-engine pacing with an honest pipeline model: pe/act/dve
        # are estimated absolute times; exp_hist holds the last two exp
        # finish times (st pool has 2 buffers, so scores wait on the exp two
        # tiles back). exp goes to ACT while that keeps pace, else DVE
        # (Schraudolph, capped), else ACT.
        sched = {"pe": 0.0, "act": 0.0, "dve": 0.0, "n_dve": 0,
                 "exp_hist": [0.0, 0.0, 0.0], "st_idx": 0,
                 "filler_w": sum(w for w, _, _, _, _ in filler),
                 "kts_left": 256.0}

        def st_depth():
            return 2 if psum_state["stp2"] is None else 3

        def alloc_st():
            i = sched["st_idx"]
            sched["st_idx"] += 1
            if K_PAIR:
                return stpair[:, (i % 2) * 1024:(i % 2 + 1) * 1024]
            if psum_state["stp2"] is not None and i % 3 == 2:
                return psum_state["stp2"].tile([128, 1024], F32, tag="st2", name="st2t")
            return stp.tile([128, 1024], F32, tag="st", name="stt")

        # ---- attention ----
        out_tiles = {}
        # deferred epilogue ops (closures): drained a few per kt during the
        # NEXT unit's early k-tiles so neither engine gets a burst at unit
        # boundaries (a DVE burst there delays that unit's first exps, which
        # stalls PE via the st-slot WAR chain)
        pending_ep = deque()

        def drain_ep(nmax):
            n = 0
            while pending_ep and n < nmax:
                eng, ns, fn = pending_ep.popleft()
                fn()
                if ns:
                    sched[eng] = max(sched[eng], sched["pe"]) + ns
                n += 1

        def attn_unit(b, h, sqc):
            reqs = [("q", b, h // 2, 2 * sqc), ("q", b, h // 2, 2 * sqc + 1)]
            reqs += [("kv", b, sc) for sc in range(NSC)]
            reqs += [("v1", b, kt) for kt in range(NKT)]
            if h % 2 == 1:
                reqs.append(("kth", b))
            require(reqs)
            qrow = (h % 2) * 64
            qt = qT[(b, h // 2)]
            kmat = kvT[b] if h % 2 == 0 else kth[b]
            q0 = sqc * 1024

            pvA = pvp.tile([128, 512], F32, tag="pv")
            pvB = pvp.tile([128, 512], F32, tag="pv")
            ex_tiles = {}
            vlast = (b == B - 1 and h == QH - 1 and sqc == NSQ - 1)
            # shorter PV lag in the final unit shortens the drain tail
            lag = K_LASTLAG if vlast else 4
            pv_pending = deque()

            def emit_pv(kt):
                ex, ex_done = ex_tiles.pop(kt)
                sched["pe"] = max(sched["pe"], ex_done) + 217.0
                for sb in range(8):
                    g = sb % 4
                    pv = pvA if sb < 4 else pvB
                    nc.tensor.matmul(
                        pv[:, g * 65:g * 65 + 65],
                        ex[:, sb * 128:(sb + 1) * 128],
                        v1[(b, kt)][:, 0:65],
                        start=(kt == 0 and g == 0),
                        stop=(kt == NKT - 1 and g == 3),
                        skip_group_check=True)

            use_pairs = K_PAIR and not vlast
            for kt in range(NKT):
                # scores wait for the exp st_depth() tiles back (slot WAR)
                if use_pairs:
                    if kt % 2 == 0:
                        sched["pe"] = (max(sched["pe"],
                                           sched["exp_hist"][-1]) + 427.0)
                    else:
                        sched["pe"] += 427.0
                else:
                    sched["pe"] = (max(sched["pe"],
                                       sched["exp_hist"][-st_depth()])
                                   + 427.0)
                st = alloc_st()
                for qc in range(2):
                    nc.tensor.matmul(
                        st[:, qc * 512:(qc + 1) * 512],
                        kmat[qrow:qrow + 64, kt * 128:(kt + 1) * 128],
                        qt[qrow:qrow + 64, q0 + qc * 512:q0 + (qc + 1) * 512],
                        start=True, stop=True)
                now = sched["pe"]
                pace = 644.0 + (427.0 if filler else 0.0)
                if use_pairs:
                    if kt % 2 == 1:
                        # merged exp over the full [128, 2048] stpair: one
                        # instruction covers both k-tiles of the pair
                        ex2 = expp.tile([128, 2048], BF16, tag="ex")
                        act_fin = max(sched["act"], now) + 1891.0
                        dve_fin = max(sched["dve"], now) + 2258.0
                        tol = 2.0 * pace
                        use_act = (act_fin <= now + tol
                                   or dve_fin > now + tol
                                   or sched["n_dve"] + 2 > K_DVECAP)
                        if use_act:
                            nc.scalar.activation(ex2[:], stpair[:], Exp,
                                                 bias=zb[:])
                            sched["act"] = act_fin
                            ex_done = act_fin
                        else:
                            nc.vector.tensor_scalar(
                                ex2[:].bitcast(I16), stpair[:],
                                SCH_A, SCH_B, Mult, Add)
                            sched["dve"] = dve_fin
                            sched["n_dve"] += 2
                            ex_done = dve_fin
                        ex_tiles[kt - 1] = (ex2[:, 0:1024], ex_done)
                        ex_tiles[kt] = (ex2[:, 1024:2048], ex_done)
                        sched["exp_hist"] = (sched["exp_hist"][-2:]
                                             + [ex_done])
                    if kt < 4:
                        drain_ep(3)
                    if kt >= lag:
                        emit_pv(kt - lag)
                    budget = K_BUDGET
                    while filler and budget > 0:
                        budget -= pop_filler()
                    continue
                ex = expp.tile([128, 1024], BF16, tag="ex")
                act_fin = max(sched["act"], now) + 1038.0
                dve_fin = max(sched["dve"], now) + 1192.0
                final_rush = (b == B - 1 and h == QH - 1 and sqc == NSQ - 1
                              and kt >= NKT - 6)
                tol = st_depth() * pace
                if final_rush:
                    use_act = act_fin <= dve_fin
                elif K_SPLIT == "static":
                    use_act = (kt % 2 == 0 or kt == NKT - 1
                               or sched["n_dve"] >= K_DVECAP)
                else:
                    use_act = (act_fin <= now + tol
                               or dve_fin > now + tol
                               or sched["n_dve"] >= K_DVECAP)
                if final_rush and K_RUSHSPLIT:
                    # drain tail: halve the last tiles' exp latency by
                    # splitting each across both engines in parallel
                    nc.scalar.activation(ex[:, 0:512], st[:, 0:512], Exp,
                                         bias=zb[:])
                    nc.vector.tensor_scalar(
                        ex[:, 512:1024].bitcast(I16), st[:, 512:1024],
                        SCH_A, SCH_B, Mult, Add)
                    sched["act"] = max(sched["act"], now) + 612.0
                    sched["dve"] = max(sched["dve"], now) + 658.0
                    ex_done = max(sched["act"], sched["dve"])
                elif K_EXPSPLIT:
                    # halve exp latency: ACT does cols 0:512 (exact exp),
                    # DVE does 512:1024 (Schraudolph) in parallel. The same
                    # sq columns use Schraudolph for every k-tile, so its
                    # multiplicative bias cancels in the softmax
                    # normalization for those output rows.
                    nc.scalar.activation(ex[:, 0:512], st[:, 0:512], Exp,
                                         bias=zb[:])
                    nc.vector.tensor_scalar(
                        ex[:, 512:1024].bitcast(I16), st[:, 512:1024],
                        SCH_A, SCH_B, Mult, Add)
                    sched["act"] = max(sched["act"], now) + 612.0
                    sched["dve"] = max(sched["dve"], now) + 658.0
                    ex_done = max(sched["act"], sched["dve"])
                elif use_act:
                    nc.scalar.activation(ex[:], st[:], Exp, bias=zb[:])
                    sched["act"] = act_fin
                    ex_done = act_fin
                else:
                    nc.vector.tensor_scalar(
                        ex[:].bitcast(I16), st[:], SCH_A, SCH_B, Mult, Add)
                    sched["dve"] = dve_fin
                    sched["n_dve"] += 1
                    ex_done = dve_fin
                ex_tiles[kt] = (ex, ex_done)
                sched["exp_hist"] = sched["exp_hist"][-2:] + [ex_done]
                if kt < 4:
                    drain_ep(3)
                if K_PVADAPT:
                    pv_pending.append(kt)
                    while pv_pending and (
                            len(pv_pending) > 4
                            or ex_tiles[pv_pending[0]][1] <= sched["pe"]):
                        emit_pv(pv_pending.popleft())
                elif kt >= lag:
                    emit_pv(kt - lag)
                budget = K_BUDGET
                while filler and budget > 0:
                    budget -= pop_filler()
                if (ENABLE_STP2 and not K_PAIR and not filler
                        and psum_state["stp2"] is None):
                    psum_state["projp_cm"].__exit__(None, None, None)
                    psum_state["stp2"] = ctx.enter_context(
                        tc.tile_pool(name="stp2", bufs=1, space="PSUM"))
            if K_PVADAPT:
                while pv_pending:
                    emit_pv(pv_pending.popleft())
            else:
                for k_ in range(NKT - lag, NKT):
                    emit_pv(k_)

            # epilogue: normalize and write output tiles. Deferred into
            # pending_ep except for the very last unit (nothing follows it).
            if b not in out_tiles:
                out_tiles[b] = outp.tile([128, 16 * MCOLS], BF16, tag="out",
                                         name=f"out{b}")
            very_last = (b == B - 1 and h == QH - 1 and sqc == NSQ - 1)
            Copy = mybir.ActivationFunctionType.Copy
            recs = []
            rec_ops = []
            for pv in (pvA, pvB):
                r4 = recp.tile([128, 4], F32, tag="rec")
                def rec_op(r4=r4, pv=pv):
                    nc.vector.reciprocal(
                        r4[:], pv[:, 0:260].rearrange(
                            "p (g c) -> p g c", c=65)[:, :, 64:65])
                rec_ops.append(("dve", 129.0, rec_op))
                recs.append(r4)
            mul_ops = []
            for sb in range(8):
                g = sb % 4
                pv = pvA if sb < 4 else pvB
                st_i = sqc * 8 + sb
                out_ap = out_tiles[b][:, st_i * MCOLS + h * 64:
                                      st_i * MCOLS + (h + 1) * 64]
                if sb % 2 == 1 and (K_EPI == "defer" or very_last):
                    def mul_op(out_ap=out_ap, pv=pv, g=g, sb=sb):
                        nc.scalar.activation(
                            out_ap, pv[:, g * 65:g * 65 + 64], Copy,
                            scale=recs[sb // 4][:, g:g + 1])
                    mul_ops.append(("act", 238.0, mul_op))
                else:
                    def mul_op(out_ap=out_ap, pv=pv, g=g, sb=sb):
                        nc.vector.tensor_scalar_mul(
                            out_ap, pv[:, g * 65:g * 65 + 64],
                            recs[sb // 4][:, g:g + 1])
                    mul_ops.append(("dve", 192.0, mul_op))
            dma_ops = []
            if b == B - 1 and h == QH - 1:
                def dma_op(sqc=sqc):
                    lo = sqc * 8
                    half = out_d[b, lo * 128:(sqc + 1) * 1024, :].rearrange(
                        "(blk p) c -> p blk c", p=128)
                    src_ap = out_tiles[b][:, lo * MCOLS:
                                          (sqc + 1) * 8 * MCOLS].rearrange(
                        "p (blk c) -> p blk c", c=MCOLS)
                    nc.sync.dma_start(out=half, in_=src_ap)
                dma_ops.append((None, 0.0, dma_op))
            elif b == 0 and h == QH - 1 and sqc == NSQ - 1:
                def dma_op():
                    nc.sync.dma_start(
                        out=out_d[0].rearrange("(blk p) c -> p blk c", p=128),
                        in_=out_tiles[0][:].rearrange(
                            "p (blk c) -> p blk c", c=MCOLS))
                dma_ops.append((None, 0.0, dma_op))
            if very_last:
                # nothing follows: run inline, quarters staged to DMA early
                for i, (eng, ns, fn) in enumerate(rec_ops + mul_ops):
                    fn()
                    if ns:
                        sched[eng] = max(sched[eng], sched["pe"]) + ns
                    if i - len(rec_ops) in (3, 5):
                        sb = i - len(rec_ops)
                        lo_t = sqc * 8 + (0 if sb == 3 else 4)
                        n_t = 4 if sb == 3 else 2
                        quarter = out_d[b, lo_t * 128:(lo_t + n_t) * 128, :]
                        nc.sync.dma_start(
                            out=quarter.rearrange("(blk p) c -> p blk c",
                                                  p=128),
                            in_=out_tiles[b][:, lo_t * MCOLS:
                                             (lo_t + n_t) * MCOLS].rearrange(
                                "p (blk c) -> p blk c", c=MCOLS))
                lo = sqc * 8 + 6
                half = out_d[b, lo * 128:(sqc + 1) * 1024, :].rearrange(
                    "(blk p) c -> p blk c", p=128)
                src_ap = out_tiles[b][:, lo * MCOLS:
                                      (sqc + 1) * 8 * MCOLS].rearrange(
                    "p (blk c) -> p blk c", c=MCOLS)
                nc.sync.dma_start(out=half, in_=src_ap)
            elif K_EPI == "inline":
                for eng, ns, fn in rec_ops + mul_ops + dma_ops:
                    fn()
                sched["dve"] = max(sched["dve"], sched["pe"]) + 2200.0
            else:
                pending_ep.extend(rec_ops)
                pending_ep.extend(mul_ops)
                pending_ep.extend(dma_ops)

        for b in range(B):
            for h in range(QH):
                for sqc in range(NSQ):
                    if (ENABLE_STP2 and not K_PAIR and not filler
                            and psum_state["stp2"] is None):
                        psum_state["projp_cm"].__exit__(None, None, None)
                        psum_state["stp2"] = ctx.enter_context(
                            tc.tile_pool(name="stp2", bufs=1, space="PSUM"))
                    attn_unit(b, h, sqc)

        # safety: drain any remaining filler and deferred epilogue ops
        while filler:
            pop_filler()
        drain_ep(len(pending_ep))
        if psum_state["stp2"] is None:
            psum_state["projp_cm"].__exit__(None, None, None)

    nc.compile()
    return nc


def make_in_maps(hidden_states, Wq, bq, Wk, bk, Wv, bv):
    bf16 = np.float16
    hs = np.asarray(hidden_states, dtype=np.float32)
    hst = np.ascontiguousarray(hs.transpose(0, 2, 1)).astype(bf16)
    Wq = np.asarray(Wq, dtype=np.float32)
    bq = np.asarray(bq, dtype=np.float32)
    Wk = np.asarray(Wk, dtype=np.float32)
    bk = np.asarray(bk, dtype=np.float32)
    Wv = np.asarray(Wv, dtype=np.float32)
    bv = np.asarray(bv, dtype=np.float32)
    sc = 1.0 / np.sqrt(np.float32(HD))
    ident = np.eye(128, dtype=np.float32).astype(bf16)
    in_maps = []
    for c in range(NCORES):
        qs = slice(c * MCOLS, (c + 1) * MCOLS)
        ks = slice(c * HD, (c + 1) * HD)
        bq_c = (bq[qs] * sc).reshape(2, 128).T
        in_maps.append({
            "hst": hst,
            "wq": np.ascontiguousarray(Wq[:, qs] * sc).astype(bf16),
            "wkv": np.ascontiguousarray(
                np.concatenate([Wk[:, ks], Wv[:, ks]], axis=1)).astype(bf16),
            "bq": np.ascontiguousarray(bq_c),
            "bkv": np.concatenate([bk[ks], bv[ks]]).reshape(128, 1),
            "ident": ident,
        })
    return in_maps


_NC_CACHE = {}


def get_nc():
    if "nc" not in _NC_CACHE:
        _NC_CACHE["nc"] = build_nc()
    return _NC_CACHE["nc"]


def kernel(hidden_states, Wq, bq, Wk, bk, Wv, bv):
    nc = get_nc()
    in_maps = make_in_maps(hidden_states, Wq, bq, Wk, bk, Wv, bv)
    res = run_bass_kernel_spmd(nc, in_maps, list(range(NCORES)))
    outs = [np.asarray(r["out"], dtype=np.float32) for r in res.results]
    return np.concatenate(outs, axis=-1)



# revision 59
# speedup vs baseline: 1.0009x; 1.0009x over previous
"""Trainium2 Bass kernel for GroupedQueryAttention (v3).

Sharding: 8 cores; core c owns KV head g=c and Q heads 4c..4c+3, both batch
elements. Each core computes its [2, 2048, 256] output slice; host concats.

Host prep: hs is transposed to hsT [B, D, S] and cast to fp16 on the host;
1/sqrt(HD) is folded into Wq/bq; weights are cast to fp16. (fp16 storage is
speed-identical to bf16 on PE/ACT/DVE but carries 3 extra mantissa bits,
roughly halving the kernel's error vs the 2e-2 budget; the DVE Schraudolph
exp constants are retuned for the fp16 bit layout.)

Per-core dataflow:
  P) Projections: Q^T (2 tiles of [128, S], head pairs), [K^T|V^T] [128, S]
     accumulate over 16 d-tiles directly from hsT (no on-device transposes).
     Startup runs 8 concurrent accumulation groups (kv + q0 + q1 pair
     passes) across all 8 PSUM banks while the serialized hsT DMA streams
     in. K^T is duplicated at partitions 64:128 (kth) for odd heads; V^T
     tiles are PE-transposed back to natural [s_k, 64] + ones column -> v1.
  A) Attention per (b, h, s_q-chunk of 1024): scores computed transposed,
     S^T [s_k=128, s_q=1024] per k-tile; exp mostly on ACT (bf16 out), a
     greedily-paced minority of k-tiles on DVE via a Schraudolph int16 bit
     trick (GPSIMD has no PSUM port); PV in natural orientation:
     ctx[s_q-block, 65] accumulates ex_chunk^T @ [V|1] over k-tiles in PSUM
     (ones column = softmax denominator); DVE does the reciprocal-scale
     epilogue into a per-batch bf16 output tile (host casts back to f32).
  Remaining projection matmuls are interleaved as paced filler inside the
  attention k-loop (with completion markers + requirement drains), keeping
  the PE continuously busy so it holds the 2.4 GHz p-state. Once the filler
  drains, the projection PSUM banks are handed to a third scores buffer,
  deepening the scores->exp->scores pipeline for the tail.

All matmul moving operands are bf16 (1 col/cycle at any output width).
"""

import sys
from collections import deque
from contextlib import ExitStack

import numpy as np

sys.path.insert(0, "/opt/trn_rl_repo")

import concourse.bass as bass  # noqa: E402
import concourse.bacc as bacc  # noqa: E402
import concourse.tile as tile  # noqa: E402
from concourse import mybir  # noqa: E402
from concourse.bass_utils import run_bass_kernel_spmd  # noqa: E402

B = 2
S = 2048
D = 2048
HD = 64
NCORES = 8
QH = 4           # q heads per core
MCOLS = QH * HD  # 256 output cols per core

BF16 = mybir.dt.float16  # fp16 storage everywhere (same speed, 8x mantissa)
F32 = mybir.dt.float32
I16 = mybir.dt.int16
Exp = mybir.ActivationFunctionType.Exp
Mult = mybir.AluOpType.mult
Add = mybir.AluOpType.add

NDT = 16         # d tiles of 128
NSC = 4          # s chunks of 512 (projection)
NKT = 16         # s_k tiles of 128
NSQ = 2          # s_q chunks of 1024

# Schraudolph exp constants (fp16-as-int16; tuned for truncating convert)
SCH_A = 1024.0 / 0.6931471805599453
SCH_B = 15360.0 - 44.0
# Max exp tiles offloaded to DVE via Schraudolph (of 256). fp16 storage of
# the exact-exp path frees error budget, so the cap can sit near the
# ACT/DVE balance point rather than the old bf16 error-driven 56.
# (GPSIMD/Pool has no PSUM port, so the offload engine must be DVE.)
DVE_EXP_CAP = 112
import os  # noqa: E402
ENABLE_STP2 = os.environ.get("K_STP2", "1") == "1"
ENABLE_KV_PAR = os.environ.get("K_KVPAR", "1") == "1"
K_DVECAP = int(os.environ.get("K_DVECAP", str(DVE_EXP_CAP)))
K_BUDGET = float(os.environ.get("K_BUDGET", "0.3"))
K_BIAS = os.environ.get("K_BIAS", "dve")      # filler bias engine
K_EPI = os.environ.get("K_EPI", "inline")     # defer | inline
K_SPLIT = os.environ.get("K_SPLIT", "greedy")  # greedy | static
K_EPIMUL = os.environ.get("K_EPIMUL", "alt")  # alt | dve (deferred muls)
K_LASTLAG = int(os.environ.get("K_LASTLAG", "4"))  # pv lag in final unit
K_EXPSPLIT = os.environ.get("K_EXPSPLIT", "0") == "1"
K_PVADAPT = os.environ.get("K_PVADAPT", "0") == "1"
K_PAIR = os.environ.get("K_PAIR", "0") == "1"
K_SUSPLIT = int(os.environ.get("K_SUSPLIT", "0"))
K_RUSHSPLIT = os.environ.get("K_RUSHSPLIT", "0") == "1"
K_GLAG = int(os.environ.get("K_GLAG", "4"))    # global pv lag
K_EXPB = int(os.environ.get("K_EXPB", "5"))    # expp pool bufs
K_HSTB = int(os.environ.get("K_HSTB", "13"))   # hstp pool bufs
K_RUSH = int(os.environ.get("K_RUSH", "8"))    # final-rush window (k-tiles)
K_PACE = float(os.environ.get("K_PACE", "1.0"))  # greedy pace multiplier


def build_nc():
    nc = bacc.Bacc("TRN2", target_bir_lowering=False, debug=False)

    hst_d = nc.dram_tensor("hst", [B, D, S], BF16, kind="ExternalInput")
    wq_d = nc.dram_tensor("wq", [D, MCOLS], BF16, kind="ExternalInput")
    wkv_d = nc.dram_tensor("wkv", [D, 128], BF16, kind="ExternalInput")
    bq_d = nc.dram_tensor("bq", [128, 2], F32, kind="ExternalInput")
    bkv_d = nc.dram_tensor("bkv", [128, 1], F32, kind="ExternalInput")
    id_d = nc.dram_tensor("ident", [128, 128], BF16, kind="ExternalInput")
    out_d = nc.dram_tensor("out", [B, S, MCOLS], BF16, kind="ExternalOutput")

    with tile.TileContext(nc) as tc, ExitStack() as ctx:
        const = ctx.enter_context(tc.tile_pool(name="const", bufs=1))
        wqp = ctx.enter_context(tc.tile_pool(name="wqp", bufs=4))
        wkvp = ctx.enter_context(tc.tile_pool(name="wkvp", bufs=1))
        hstp = ctx.enter_context(tc.tile_pool(name="hstp", bufs=K_HSTB))
        qtp = ctx.enter_context(tc.tile_pool(name="qtp", bufs=4))
        kvp = ctx.enter_context(tc.tile_pool(name="kvp", bufs=2))
        kthp = ctx.enter_context(tc.tile_pool(name="kthp", bufs=2))
        v1p = ctx.enter_context(tc.tile_pool(name="v1p", bufs=2 * NKT))
        expp = ctx.enter_context(tc.tile_pool(name="expp", bufs=K_EXPB))
        recp = ctx.enter_context(tc.tile_pool(name="recp", bufs=4))
        outp = ctx.enter_context(tc.tile_pool(name="outp", bufs=2))
        if K_PAIR:
            # one [128, 2048] PSUM tile = both score slots, PSUM-contiguous,
            # so one merged exp instruction can read a k-tile PAIR: ACT cost
            # 1891ns per pair = 946ns/tile, below the ~970ns/kt PE cadence
            # (single [128,1024] exp costs 1038 and ACT falls behind)
            stp = ctx.enter_context(tc.tile_pool(name="stp", bufs=1,
                                                 space="PSUM"))
        else:
            stp = ctx.enter_context(tc.tile_pool(name="stp", bufs=2,
                                                 space="PSUM"))
        pvp = ctx.enter_context(tc.tile_pool(name="pvp", bufs=2, space="PSUM"))
        # projp's 2 banks are handed over to a third scores buffer (stp2)
        # once all projection work has been emitted — the deeper scores
        # pipeline hides the scores->exp->scores latency chain in the tail
        projp_cm = tc.tile_pool(name="projp", bufs=2, space="PSUM")
        projp = projp_cm.__enter__()
        psum_state = {"projp_cm": projp_cm, "stp2": None}

        # All DMA transfers serialize on one HWDGE device in the cost model,
        # in dispatch order — so put everything on one queue in exactly the
        # order the startup consumes it: (wkv_dt, hsT0_dt) pairs gate the kv
        # passes, then wq lands just in time for the q0 pass, then hsT b1.
        # consolidated loads: HWDGE charges ~630ns per DMA instruction, so
        # batch the weight/activation streams into few wide transfers
        # startup-latency split (K_SUSPLIT): dispatch small first slices of
        # wkv (and optionally hsT dt0) so the first kv matmul starts earlier;
        # each split costs one extra serialized HWDGE dispatch (~630ns)
        hsT = {}
        wkv_big = wkvp.tile([128, 16 * 128], BF16, tag="wkv", name="wkvbig")
        if K_SUSPLIT >= 1:
            nc.sync.dma_start(out=wkv_big[:, 0:128], in_=wkv_d[0:128, :])
            if K_SUSPLIT >= 2:
                t0 = hstp.tile([128, 2 * S], BF16, tag="hst", name="hsT0_0")
                nc.sync.dma_start(out=t0[:, 0:S], in_=hst_d[0, 0:128, :])
            nc.sync.dma_start(
                out=wkv_big[:, 128:].rearrange("p (blk c) -> p blk c", c=128),
                in_=wkv_d[128:, :].rearrange("(blk p) c -> p blk c", p=128))
            if K_SUSPLIT >= 2:
                nc.sync.dma_start(out=t0[:, S:2 * S], in_=hst_d[0, 128:256, :])
                hsT[(0, 0)] = t0[:, 0:S]
                hsT[(0, 1)] = t0[:, S:2 * S]
        else:
            nc.sync.dma_start(
                out=wkv_big[:].rearrange("p (blk c) -> p blk c", c=128),
                in_=wkv_d[:].rearrange("(blk p) c -> p blk c", p=128))
        wkv_sb = [wkv_big[:, dt_ * 128:(dt_ + 1) * 128] for dt_ in range(NDT)]

        wq_sb = []
        wq4 = []
        for q4 in range(4):
            wt = wqp.tile([128, 4 * MCOLS], BF16, tag="wq", name=f"wq4_{q4}")
            wq4.append(wt)
        for dt_ in range(NDT):
            if dt_ % 2 == 0 and (dt_ >= 2 or (0, 0) not in hsT):
                t = hstp.tile([128, 2 * S], BF16, tag="hst",
                              name=f"hsT0_{dt_}")
                nc.sync.dma_start(
                    out=t[:].rearrange("p (two s) -> p two s", s=S),
                    in_=hst_d[0, dt_ * 128:(dt_ + 2) * 128, :].rearrange(
                        "(two p) s -> p two s", p=128))
                hsT[(0, dt_)] = t[:, 0:S]
                hsT[(0, dt_ + 1)] = t[:, S:2 * S]
            if dt_ % 4 == 3:
                q4i = dt_ // 4
                nc.sync.dma_start(
                    out=wq4[q4i][:].rearrange("p (blk c) -> p blk c", c=MCOLS),
                    in_=wq_d[q4i * 512:(q4i + 1) * 512, :].rearrange(
                        "(blk p) c -> p blk c", p=128))
        wq_sb = [wq4[dt_ // 4][:, (dt_ % 4) * MCOLS:(dt_ % 4 + 1) * MCOLS]
                 for dt_ in range(NDT)]

        # consts dispatch from the scalar queue so the sync queue's first
        # dispatches are the startup-critical wkv/hsT tiles
        ident = const.tile([128, 128], BF16, tag="ident")
        nc.scalar.dma_start(out=ident[:], in_=id_d[:])
        bq_sb = const.tile([128, 2], F32, tag="bq")
        nc.scalar.dma_start(out=bq_sb[:], in_=bq_d[:])
        bkv_sb = const.tile([128, 1], F32, tag="bkv")
        nc.scalar.dma_start(out=bkv_sb[:], in_=bkv_d[:])
        zb = const.tile([128, 1], F32, tag="zb")
        nc.vector.memset(zb[:], 0.0)


        qT = {}   # (b, pair) -> [128, S] bf16
        kvT = {}  # b -> [128, S] bf16 (rows 0:64 K^T, 64:128 V^T)
        kth = {}  # b -> [128, S] bf16 (rows 64:128 K^T copy)
        v1 = {}   # (b, kt) -> [128, 65] bf16 ([V | 1])
        for b in range(B):
            kvT[b] = kvp.tile([128, S], BF16, tag="kv", name=f"kvT{b}")
            kth[b] = kthp.tile([128, S], BF16, tag="kth", name=f"kth{b}")
            for pair in range(2):
                qT[(b, pair)] = qtp.tile([128, S], BF16, tag="qt",
                                         name=f"qT{b}_{pair}")

        # ---- projection pass machinery ----
        # Steps are (weight, fn): weight ~ PE-engine cost in units of one
        # 512-wide matmul pair (427 ns); the attention loop consumes ~1.0
        # of weight per k-tile so cheap steps get batched.
        Ident = mybir.ActivationFunctionType.Identity

        def proj_pass_steps(b, kind, sc0, sc1, pool=None, aps=None,
                            bias_eng="dve"):
            """One pair-pass: two accumulation groups (s-chunks sc0, sc1).
            kind: 'kv' or ('q', qc). aps: explicit PSUM APs to accumulate
            into (bank-disjoint halves of donor tiles). bias_eng='act' puts
            the PSUM->SBUF bias-add on the ACT engine."""
            if aps is not None:
                psA, psB = aps
            else:
                pool = pool or projp
                tg = {id(stp): "st", id(pvp): "pv"}.get(id(pool), "pj")
                psA = pool.tile([128, 512], F32, tag=tg, name="pjA")
                psB = pool.tile([128, 512], F32, tag=tg, name="pjB")
            for dt_ in range(NDT):
                def mm_step(dt_=dt_, psA=psA, psB=psB):
                    for ps, sc in ((psA, sc0), (psB, sc1)):
                        rhs = hsT[(b, dt_)][:, sc * 512:(sc + 1) * 512]
                        if kind == "kv":
                            lhsT = wkv_sb[dt_][:]
                        else:
                            qc = kind[1]
                            lhsT = wq_sb[dt_][:, qc * 128:(qc + 1) * 128]
                        nc.tensor.matmul(ps[:], lhsT, rhs,
                                         start=(dt_ == 0), stop=(dt_ == NDT - 1))
                yield (1.0, None, 0.0, (), mm_step)

            def bias_step():
                for ps, sc in ((psA, sc0), (psB, sc1)):
                    c0, c1 = sc * 512, (sc + 1) * 512
                    if kind == "kv":
                        out_ap, b_ap = kvT[b][:, c0:c1], bkv_sb[:]
                    else:
                        qc = kind[1]
                        out_ap, b_ap = (qT[(b, qc)][:, c0:c1],
                                        bq_sb[:, qc:qc + 1])
                    if bias_eng == "act":
                        nc.scalar.activation(out_ap, ps[:], Ident, bias=b_ap)
                    else:
                        nc.vector.tensor_scalar_add(out_ap, ps[:], b_ap)
            if kind == "kv":
                marks = (("kv", b, sc0), ("kv", b, sc1))
            else:
                marks = (("q", b, kind[1], sc0), ("q", b, kind[1], sc1))
            yield (0.2, bias_eng, 1224.0 if bias_eng == "act" else 1320.0,
                   marks, bias_step)

        def vt_steps(b, copy_eng="dve"):
            """PE-transpose V^T tiles to natural [s_k, 64] + ones column."""
            for kt in range(NKT):
                def step(kt=kt):
                    pst = projp.tile([128, 64], BF16, tag="pj")
                    nc.tensor.transpose(
                        pst[:], kvT[b][64:128, kt * 128:(kt + 1) * 128],
                        ident[64:128, 64:128])
                    v = v1p.tile([128, 65], BF16, tag="v1", name=f"v1_{b}_{kt}")
                    if copy_eng == "act":
                        nc.scalar.activation(
                            v[:, 0:64], pst[:],
                            mybir.ActivationFunctionType.Copy)
                    else:
                        nc.vector.tensor_copy(v[:, 0:64], pst[:])
                    nc.gpsimd.memset(v[:, 64:65], 1.0)
                    v1[(b, kt)] = v
                yield (0.2, copy_eng, 238.0 if copy_eng == "act" else 160.0,
                       (("v1", b, kt),), step)

        def kth_step(b):
            def step():
                nc.sync.dma_start(out=kth[b][64:128, :], in_=kvT[b][0:64, :])
            yield (0.1, None, 0.0, (("kth", b),), step)

        def proj_stream(b):
            yield from proj_pass_steps(b, "kv", 0, 1, bias_eng=K_BIAS)
            yield from proj_pass_steps(b, "kv", 2, 3, bias_eng=K_BIAS)
            yield from kth_step(b)
            yield from vt_steps(b)
            yield from proj_pass_steps(b, ("q", 0), 0, 1, bias_eng=K_BIAS)
            yield from proj_pass_steps(b, ("q", 0), 2, 3, bias_eng=K_BIAS)
            yield from proj_pass_steps(b, ("q", 1), 0, 1, bias_eng=K_BIAS)
            yield from proj_pass_steps(b, ("q", 1), 2, 3, bias_eng=K_BIAS)

        # ---- batch 0 minimal prologue: kv + vt + q0 cols 0:1024; the rest
        # feeds the attention loop as filler. The two kv pair-passes run
        # concurrently (kv23 borrows the still-idle scores pool's banks) so
        # both track the serialized hsT DMA stream. ----
        done = set()
        # 8 concurrent accumulation groups during the DMA-bound startup:
        # kv01 in projp, q0-01 in pvp, and kv23 + q1-01 packed into the two
        # stp slots (bank-disjoint halves of [128,1024] donor tiles)
        if K_PAIR:
            stpair = stp.tile([128, 2048], F32, tag="st", name="stpair")
            stA = stpair[:, 0:1024]
            stB = stpair[:, 1024:2048]
        else:
            stpair = None
            stA = stp.tile([128, 1024], F32, tag="st", name="stdonA")
            stB = stp.tile([128, 1024], F32, tag="st", name="stdonB")
        streams = [
            proj_pass_steps(0, "kv", 0, 1, bias_eng="act"),
            proj_pass_steps(0, "kv", 2, 3,
                            aps=(stA[:, 0:512], stB[:, 0:512]),
                            bias_eng="act"),
            proj_pass_steps(0, ("q", 0), 0, 1, pool=pvp),
            proj_pass_steps(0, ("q", 1), 0, 1,
                            aps=(stA[:, 512:1024], stB[:, 512:1024]),
                            bias_eng="act"),
        ]
        for steps in zip(*streams):
            for _, _, _, m, s in steps:
                s()
                done.update(m)
        for _, _, _, m, step in kth_step(0):
            step()
            done.update(m)
        for _, _, _, m, step in vt_steps(0):
            step()
            done.update(m)

        for dt_ in range(0, NDT, 2):
            t = hstp.tile([128, 2 * S], BF16, tag="hst", name=f"hsT1_{dt_}")
            nc.sync.dma_start(
                out=t[:].rearrange("p (two s) -> p two s", s=S),
                in_=hst_d[1, dt_ * 128:(dt_ + 2) * 128, :].rearrange(
                    "(two p) s -> p two s", p=128))
            hsT[(1, dt_)] = t[:, 0:S]
            hsT[(1, dt_ + 1)] = t[:, S:2 * S]

        filler = deque()
        filler.extend(proj_pass_steps(0, ("q", 0), 2, 3, bias_eng=K_BIAS))
        filler.extend(proj_pass_steps(0, ("q", 1), 2, 3, bias_eng=K_BIAS))
        filler.extend(proj_stream(1))

        def pop_filler():
            w, eng, eng_ns, marks, fn = filler.popleft()
            fn()
            done.update(marks)
            sched["pe"] += w * 427.0
            sched["filler_w"] -= w
            if eng_ns:
                sched[eng] = max(sched[eng], sched["pe"]) + eng_ns
            return w

        def require(reqs):
            while filler and not all(r in done for r in reqs):
                pop_filler()

        # Greedy per-engine pacing with an honest pipeline model: pe/act/dve
        # are estimated absolute times; exp_hist holds the last two exp
        # finish times (st pool has 2 buffers, so scores wait on the exp two
        # tiles back). exp goes to ACT while that keeps pace, else DVE
        # (Schraudolph, capped), else ACT.
        sched = {"pe": 0.0, "act": 0.0, "dve": 0.0, "n_dve": 0,
                 "exp_hist": [0.0, 0.0, 0.0], "st_idx": 0,
                 "filler_w": sum(w for w, _, _, _, _ in filler),
                 "kts_left": 256.0}

        def st_depth():
            return 2 if psum_state["stp2"] is None else 3

        def alloc_st():
            i = sched["st_idx"]
            sched["st_idx"] += 1
            if K_PAIR:
                return stpair[:, (i % 2) * 1024:(i % 2 + 1) * 1024]
            if psum_state["stp2"] is not None and i % 3 == 2:
                return psum_state["stp2"].tile([128, 1024], F32, tag="st2", name="st2t")
            return stp.tile([128, 1024], F32, tag="st", name="stt")

        # ---- attention ----
        out_tiles = {}
        # deferred epilogue ops (closures): drained a few per kt during the
        # NEXT unit's early k-tiles so neither engine gets a burst at unit
        # boundaries (a DVE burst there delays that unit's first exps, which
        # stalls PE via the st-slot WAR chain)
        pending_ep = deque()

        def drain_ep(nmax):
            n = 0
            while pending_ep and n < nmax:
                eng, ns, fn = pending_ep.popleft()
                fn()
                if ns:
                    sched[eng] = max(sched[eng], sched["pe"]) + ns
                n += 1

        def attn_unit(b, h, sqc):
            reqs = [("q", b, h // 2, 2 * sqc), ("q", b, h // 2, 2 * sqc + 1)]
            reqs += [("kv", b, sc) for sc in range(NSC)]
            reqs += [("v1", b, kt) for kt in range(NKT)]
            if h % 2 == 1:
                reqs.append(("kth", b))
            require(reqs)
            qrow = (h % 2) * 64
            qt = qT[(b, h // 2)]
            kmat = kvT[b] if h % 2 == 0 else kth[b]
            q0 = sqc * 1024

            pvA = pvp.tile([128, 512], F32, tag="pv")
            pvB = pvp.tile([128, 512], F32, tag="pv")
            ex_tiles = {}
            vlast = (b == B - 1 and h == QH - 1 and sqc == NSQ - 1)
            # shorter PV lag in the final unit shortens the drain tail
            lag = K_LASTLAG if vlast else K_GLAG
            pv_pending = deque()

            def emit_pv(kt):
                ex, ex_done = ex_tiles.pop(kt)
                sched["pe"] = max(sched["pe"], ex_done) + 217.0
                for sb in range(8):
                    g = sb % 4
                    pv = pvA if sb < 4 else pvB
                    nc.tensor.matmul(
                        pv[:, g * 65:g * 65 + 65],
                        ex[:, sb * 128:(sb + 1) * 128],
                        v1[(b, kt)][:, 0:65],
                        start=(kt == 0 and g == 0),
                        stop=(kt == NKT - 1 and g == 3),
                        skip_group_check=True)

            use_pairs = K_PAIR and not vlast
            for kt in range(NKT):
                # scores wait for the exp st_depth() tiles back (slot WAR)
                if use_pairs:
                    if kt % 2 == 0:
                        sched["pe"] = (max(sched["pe"],
                                           sched["exp_hist"][-1]) + 427.0)
                    else:
                        sched["pe"] += 427.0
                else:
                    sched["pe"] = (max(sched["pe"],
                                       sched["exp_hist"][-st_depth()])
                                   + 427.0)
                st = alloc_st()
                for qc in range(2):
                    nc.tensor.matmul(
                        st[:, qc * 512:(qc + 1) * 512],
                        kmat[qrow:qrow + 64, kt * 128:(kt + 1) * 128],
                        qt[qrow:qrow + 64, q0 + qc * 512:q0 + (qc + 1) * 512],
                        start=True, stop=True)
                now = sched["pe"]
                pace = (644.0 + (427.0 if filler else 0.0)) * K_PACE
                if use_pairs:
                    if kt % 2 == 1:
                        # merged exp over the full [128, 2048] stpair: one
                        # instruction covers both k-tiles of the pair
                        ex2 = expp.tile([128, 2048], BF16, tag="ex")
                        act_fin = max(sched["act"], now) + 1891.0
                        dve_fin = max(sched["dve"], now) + 2258.0
                        tol = 2.0 * pace
                        use_act = (act_fin <= now + tol
                                   or dve_fin > now + tol
                                   or sched["n_dve"] + 2 > K_DVECAP)
                        if use_act:
                            nc.scalar.activation(ex2[:], stpair[:], Exp,
                                                 bias=zb[:])
                            sched["act"] = act_fin
                            ex_done = act_fin
                        else:
                            nc.vector.tensor_scalar(
                                ex2[:].bitcast(I16), stpair[:],
                                SCH_A, SCH_B, Mult, Add)
                            sched["dve"] = dve_fin
                            sched["n_dve"] += 2
                            ex_done = dve_fin
                        ex_tiles[kt - 1] = (ex2[:, 0:1024], ex_done)
                        ex_tiles[kt] = (ex2[:, 1024:2048], ex_done)
                        sched["exp_hist"] = (sched["exp_hist"][-2:]
                                             + [ex_done])
                    if kt < 4:
                        drain_ep(3)
                    if kt >= lag:
                        emit_pv(kt - lag)
                    budget = K_BUDGET
                    while filler and budget > 0:
                        budget -= pop_filler()
                    continue
                ex = expp.tile([128, 1024], BF16, tag="ex")
                act_fin = max(sched["act"], now) + 1038.0
                dve_fin = max(sched["dve"], now) + 1192.0
                unit_idx = (b * QH + h) * NSQ + sqc
                kts_from_end = (B * QH * NSQ - 1 - unit_idx) * NKT + NKT - kt
                final_rush = kts_from_end <= K_RUSH
                tol = st_depth() * pace
                if final_rush:
                    use_act = act_fin <= dve_fin
                elif K_SPLIT == "static":
                    use_act = (kt % 2 == 0 or kt == NKT - 1
                               or sched["n_dve"] >= K_DVECAP)
                else:
                    use_act = (act_fin <= now + tol
                               or dve_fin > now + tol
                               or sched["n_dve"] >= K_DVECAP)
                if final_rush and K_RUSHSPLIT:
                    # drain tail: halve the last tiles' exp latency by
                    # splitting each across both engines in parallel
                    nc.scalar.activation(ex[:, 0:512], st[:, 0:512], Exp,
                                         bias=zb[:])
                    nc.vector.tensor_scalar(
                        ex[:, 512:1024].bitcast(I16), st[:, 512:1024],
                        SCH_A, SCH_B, Mult, Add)
                    sched["act"] = max(sched["act"], now) + 612.0
                    sched["dve"] = max(sched["dve"], now) + 658.0
                    ex_done = max(sched["act"], sched["dve"])
                elif K_EXPSPLIT:
                    # halve exp latency: ACT does cols 0:512 (exact exp),
                    # DVE does 512:1024 (Schraudolph) in parallel. The same
                    # sq columns use Schraudolph for every k-tile, so its
                    # multiplicative bias cancels in the softmax
                    # normalization for those output rows.
                    nc.scalar.activation(ex[:, 0:512], st[:, 0:512], Exp,
                                         bias=zb[:])
                    nc.vector.tensor_scalar(
                        ex[:, 512:1024].bitcast(I16), st[:, 512:1024],
                        SCH_A, SCH_B, Mult, Add)
                    sched["act"] = max(sched["act"], now) + 612.0
                    sched["dve"] = max(sched["dve"], now) + 658.0
                    ex_done = max(sched["act"], sched["dve"])
                elif use_act:
                    nc.scalar.activation(ex[:], st[:], Exp, bias=zb[:])
                    sched["act"] = act_fin
                    ex_done = act_fin
                else:
                    nc.vector.tensor_scalar(
                        ex[:].bitcast(I16), st[:], SCH_A, SCH_B, Mult, Add)
                    sched["dve"] = dve_fin
                    sched["n_dve"] += 1
                    ex_done = dve_fin
                ex_tiles[kt] = (ex, ex_done)
                sched["exp_hist"] = sched["exp_hist"][-2:] + [ex_done]
                if kt < 4:
                    drain_ep(3)
                if K_PVADAPT:
                    pv_pending.append(kt)
                    while pv_pending and (
                            len(pv_pending) > 4
                            or ex_tiles[pv_pending[0]][1] <= sched["pe"]):
                        emit_pv(pv_pending.popleft())
                elif kt >= lag:
                    emit_pv(kt - lag)
                budget = K_BUDGET
                while filler and budget > 0:
                    budget -= pop_filler()
                if (ENABLE_STP2 and not K_PAIR and not filler
                        and psum_state["stp2"] is None):
                    psum_state["projp_cm"].__exit__(None, None, None)
                    psum_state["stp2"] = ctx.enter_context(
                        tc.tile_pool(name="stp2", bufs=1, space="PSUM"))
            if K_PVADAPT:
                while pv_pending:
                    emit_pv(pv_pending.popleft())
            else:
                for k_ in range(NKT - lag, NKT):
                    emit_pv(k_)

            # epilogue: normalize and write output tiles. Deferred into
            # pending_ep except for the very last unit (nothing follows it).
            if b not in out_tiles:
                out_tiles[b] = outp.tile([128, 16 * MCOLS], BF16, tag="out",
                                         name=f"out{b}")
            very_last = (b == B - 1 and h == QH - 1 and sqc == NSQ - 1)
            Copy = mybir.ActivationFunctionType.Copy
            recs = []
            rec_ops = []
            for pv in (pvA, pvB):
                r4 = recp.tile([128, 4], F32, tag="rec")
                def rec_op(r4=r4, pv=pv):
                    nc.vector.reciprocal(
                        r4[:], pv[:, 0:260].rearrange(
                            "p (g c) -> p g c", c=65)[:, :, 64:65])
                rec_ops.append(("dve", 129.0, rec_op))
                recs.append(r4)
            mul_ops = []
            for sb in range(8):
                g = sb % 4
                pv = pvA if sb < 4 else pvB
                st_i = sqc * 8 + sb
                out_ap = out_tiles[b][:, st_i * MCOLS + h * 64:
                                      st_i * MCOLS + (h + 1) * 64]
                if sb % 2 == 1 and (K_EPI == "defer" or very_last):
                    def mul_op(out_ap=out_ap, pv=pv, g=g, sb=sb):
                        nc.scalar.activation(
                            out_ap, pv[:, g * 65:g * 65 + 64], Copy,
                            scale=recs[sb // 4][:, g:g + 1])
                    mul_ops.append(("act", 238.0, mul_op))
                else:
                    def mul_op(out_ap=out_ap, pv=pv, g=g, sb=sb):
                        nc.vector.tensor_scalar_mul(
                            out_ap, pv[:, g * 65:g * 65 + 64],
                            recs[sb // 4][:, g:g + 1])
                    mul_ops.append(("dve", 192.0, mul_op))
            dma_ops = []
            if b == B - 1 and h == QH - 1:
                def dma_op(sqc=sqc):
                    lo = sqc * 8
                    half = out_d[b, lo * 128:(sqc + 1) * 1024, :].rearrange(
                        "(blk p) c -> p blk c", p=128)
                    src_ap = out_tiles[b][:, lo * MCOLS:
                                          (sqc + 1) * 8 * MCOLS].rearrange(
                        "p (blk c) -> p blk c", c=MCOLS)
                    nc.sync.dma_start(out=half, in_=src_ap)
                dma_ops.append((None, 0.0, dma_op))
            elif b == 0 and h == QH - 1 and sqc == NSQ - 1:
                def dma_op():
                    nc.sync.dma_start(
                        out=out_d[0].rearrange("(blk p) c -> p blk c", p=128),
                        in_=out_tiles[0][:].rearrange(
                            "p (blk c) -> p blk c", c=MCOLS))
                dma_ops.append((None, 0.0, dma_op))
            if very_last:
                # nothing follows: run inline, quarters staged to DMA early
                for i, (eng, ns, fn) in enumerate(rec_ops + mul_ops):
                    fn()
                    if ns:
                        sched[eng] = max(sched[eng], sched["pe"]) + ns
                    if i - len(rec_ops) in (3, 5):
                        sb = i - len(rec_ops)
                        lo_t = sqc * 8 + (0 if sb == 3 else 4)
                        n_t = 4 if sb == 3 else 2
                        quarter = out_d[b, lo_t * 128:(lo_t + n_t) * 128, :]
                        nc.sync.dma_start(
                            out=quarter.rearrange("(blk p) c -> p blk c",
                                                  p=128),
                            in_=out_tiles[b][:, lo_t * MCOLS:
                                             (lo_t + n_t) * MCOLS].rearrange(
                                "p (blk c) -> p blk c", c=MCOLS))
                lo = sqc * 8 + 6
                half = out_d[b, lo * 128:(sqc + 1) * 1024, :].rearrange(
                    "(blk p) c -> p blk c", p=128)
                src_ap = out_tiles[b][:, lo * MCOLS:
                                      (sqc + 1) * 8 * MCOLS].rearrange(
                    "p (blk c) -> p blk c", c=MCOLS)
                nc.sync.dma_start(out=half, in_=src_ap)
            elif K_EPI == "inline":
                for eng, ns, fn in rec_ops + mul_ops + dma_ops:
                    fn()
                sched["dve"] = max(sched["dve"], sched["pe"]) + 2200.0
            else:
                pending_ep.extend(rec_ops)
                pending_ep.extend(mul_ops)
                pending_ep.extend(dma_ops)

        for b in range(B):
            for h in range(QH):
                for sqc in range(NSQ):
                    if (ENABLE_STP2 and not K_PAIR and not filler
                            and psum_state["stp2"] is None):
                        psum_state["projp_cm"].__exit__(None, None, None)
                        psum_state["stp2"] = ctx.enter_context(
                            tc.tile_pool(name="stp2", bufs=1, space="PSUM"))
                    attn_unit(b, h, sqc)

        # safety: drain any remaining filler and deferred epilogue ops
        while filler:
            pop_filler()
        drain_ep(len(pending_ep))
        if psum_state["stp2"] is None:
            psum_state["projp_cm"].__exit__(None, None, None)

    nc.compile()
    return nc


def make_in_maps(hidden_states, Wq, bq, Wk, bk, Wv, bv):
    bf16 = np.float16
    hs = np.asarray(hidden_states, dtype=np.float32)
    hst = np.ascontiguousarray(hs.transpose(0, 2, 1)).astype(bf16)
    Wq = np.asarray(Wq, dtype=np.float32)
    bq = np.asarray(bq, dtype=np.float32)
    Wk = np.asarray(Wk, dtype=np.float32)
    bk = np.asarray(bk, dtype=np.float32)
    Wv = np.asarray(Wv, dtype=np.float32)
    bv = np.asarray(bv, dtype=np.float32)
    sc = 1.0 / np.sqrt(np.float32(HD))
    ident = np.eye(128, dtype=np.float32).astype(bf16)
    in_maps = []
    for c in range(NCORES):
        qs = slice(c * MCOLS, (c + 1) * MCOLS)
        ks = slice(c * HD, (c + 1) * HD)
        bq_c = (bq[qs] * sc).reshape(2, 128).T
        in_maps.append({
            "hst": hst,
            "wq": np.ascontiguousarray(Wq[:, qs] * sc).astype(bf16),
            "wkv": np.ascontiguousarray(
                np.concatenate([Wk[:, ks], Wv[:, ks]], axis=1)).astype(bf16),
            "bq": np.ascontiguousarray(bq_c),
            "bkv": np.concatenate([bk[ks], bv[ks]]).reshape(128, 1),
            "ident": ident,
        })
    return in_maps


_NC_CACHE = {}


def get_nc():
    if "nc" not in _NC_CACHE:
        _NC_CACHE["nc"] = build_nc()
    return _NC_CACHE["nc"]


def kernel(hidden_states, Wq, bq, Wk, bk, Wv, bv):
    nc = get_nc()
    in_maps = make_in_maps(hidden_states, Wq, bq, Wk, bk, Wv, bv)
    res = run_bass_kernel_spmd(nc, in_maps, list(range(NCORES)))
    outs = [np.asarray(r["out"], dtype=np.float32) for r in res.results]
    return np.concatenate(outs, axis=-1)

